# revision 25
# baseline (speedup 1.0000x reference)
"""Trainium2 Bass kernel for nn_AutoEncoderLSTM (H=512, F=64, B=128, T=512).

Strategy: data-parallel over batch (16 rows/core on 8 cores). The LSTM
recurrence is weight-streaming bound on the PE: per step the weights are the
moving operand (1 col/cycle in fp32r at N=512) against a tiny stationary
[hidden^T x batch16] tile. The decoder feeds its own output back as input
(x_t == h_t for t>=1) so W_ih_dec + W_hh_dec are pre-summed, halving decoder
matmul work. Gates are computed with weight rows permuted to [i|f|o|g] per
128-hidden block so the epilogue needs only 2 ACT ops per psum pair.
"""

import os

import numpy as np

import concourse.bacc as bacc
import concourse.bass as bass
import concourse.mybir as mybir
import concourse.tile as tile
from concourse.bass_utils import run_bass_kernel_spmd

ds = bass.ds
F32 = mybir.dt.float32
AF = mybir.ActivationFunctionType

H = 512
F = 64
B = 128
NCORES = 8
BL = B // NCORES          # batch per core
GROUP = 8                 # steps per loop body
T = int(os.environ.get("KERNEL_T", "512"))
NG = T // GROUP
MM_DT = getattr(mybir.dt, os.environ.get("KERNEL_MM_DT", "float32r"))

_CACHE = {}
LAST_RESULTS = None
_TRACE_HOOK_DONE = False


def _install_trace_hook():
    """The agent image's antenv lacks axon_hooks; inject it and register the
    ctypes NTFF hook from trn_agent_boot so trace=True works under axon."""
    global _TRACE_HOOK_DONE
    if _TRACE_HOOK_DONE:
        return
    import sys
    import types

    import antenv
    import concourse.bass_utils as bu

    mod = types.ModuleType("antenv.axon_hooks")
    mod._hook = None
    mod.set_axon_ntff_profile_hook = lambda h: setattr(mod, "_hook", h)
    mod.get_axon_ntff_profile_hook = lambda: mod._hook
    sys.modules["antenv.axon_hooks"] = mod
    antenv.axon_hooks = mod
    from trn_agent_boot.trn_boot import _ntff_profile_via_ctypes
    mod._hook = _ntff_profile_via_ctypes("/opt/axon/libaxon_pjrt.so")
    bu.upload_artifacts = lambda tmpdir: "local://" + tmpdir
    _TRACE_HOOK_DONE = True


def _gate_perm():
    """new_col -> old_row permutation of the 4H gate dim.

    Torch gate order is rows [i(512) f(512) g(512) o(512)].  New layout:
    block n in 0..3 -> [i_n(128) f_n(128) o_n(128) g_n(128)].
    """
    perm = np.empty(4 * H, dtype=np.int64)
    for n in range(4):
        base = n * 512
        perm[base + 0:base + 128] = np.arange(0 * H + n * 128, 0 * H + n * 128 + 128)
        perm[base + 128:base + 256] = np.arange(1 * H + n * 128, 1 * H + n * 128 + 128)
        perm[base + 256:base + 384] = np.arange(3 * H + n * 128, 3 * H + n * 128 + 128)
        perm[base + 384:base + 512] = np.arange(2 * H + n * 128, 2 * H + n * 128 + 128)
    return perm


def _r(ap):
    return ap.bitcast(MM_DT)


def _emit_step_x(nc, pools, state, r, xstage, dec_bias):
    pgates = pools["pgates"]
    pg = [pgates.tile([BL, 1024], F32, name="pg0", tag="pg"),
          pgates.tile([BL, 1024], F32, name="pg1", tag="pg")]
    for n in range(4):
        out = pg[n // 2][:, (n % 2) * 512:(n % 2 + 1) * 512]
        if xstage is not None:
            nc.tensor.matmul(out, _r(xstage[:, 16 * r:16 * r + 16]),
                             _r(state["wxe"][:, n * 512:(n + 1) * 512]),
                             start=True, stop=False)
        else:
            ones_t, bd_t = dec_bias
            nc.tensor.matmul(out, _r(ones_t[:]),
                             _r(bd_t[:, n * 512:(n + 1) * 512]),
                             start=True, stop=False)
    state["pg_pending"] = pg


def _emit_step(nc, tc, r, pools, state, moving_h, xstage, dec_bias, nxt=None):
    """One LSTM step.  r = step index within the 8-step group.

    moving_h: SBUF (128, 4, 2048) weight tile streamed against the 4 h^T
       chunks of the previous step (taken from hist slot 8-r mod 8).
    xstage: encoder (65, 128) [x^T; ones] tile (cols 16r:16r+16) or None.
    dec_bias: (ones_tile, bd_tile) for decoder bias matmul, or None.
    """
    pgates, ptr, gpool, mpool, tcpool = (
        pools["pgates"], pools["ptr"], pools["g"], pools["m"], pools["tc"])
    h_t, c_t, hist, ident = state["h"], state["c"], state["hist"], state["ident"]

    slot_prev = (8 - r) % 8          # hist slot holding h^T of step t-1
    slot_cur = 7 - r

    pg = state["pg_pending"]
    histv = hist.rearrange("p (k s b) -> p k s b", k=4, s=8)

    def out_ap(n):
        return pg[n // 2][:, (n % 2) * 512:(n % 2 + 1) * 512]

    for p in range(2):
        for k in range(4):
            lhsT = histv[:, k, slot_prev, :]
            for n in (2 * p, 2 * p + 1):
                nc.tensor.matmul(out_ap(n), _r(lhsT),
                                 _r(moving_h[:, k, n * 512:(n + 1) * 512]),
                                 start=False, stop=(k == 3))
    if nxt is not None:
        _emit_step_x(nc, pools, state, *nxt)

    # Activations: per psum pair, sigmoid over [i,f,o] and tanh over [g].
    g_sb = gpool.tile([BL, 2048], F32)
    g4 = g_sb.rearrange("b (c x) -> b c x", c=4)
    for p in range(2):
        pg2 = pg[p].rearrange("b (c x) -> b c x", c=2)
        nc.scalar.activation(g4[:, 2 * p:2 * p + 2, 0:384], pg2[:, :, 0:384],
                             AF.Sigmoid)
        nc.scalar.activation(g4[:, 2 * p:2 * p + 2, 384:512], pg2[:, :, 384:512],
                             AF.Tanh)

    si = g4[:, :, 0:128]
    sf = g4[:, :, 128:256]
    so = g4[:, :, 256:384]
    tg = g4[:, :, 384:512]
    c4 = c_t.rearrange("b (k x) -> b k x", k=4)
    h4 = h_t.rearrange("b (k x) -> b k x", k=4)

    m_t = mpool.tile([BL, 4, 128], F32)
    nc.vector.tensor_mul(m_t[:], si, tg)
    nc.vector.tensor_mul(c4, sf, c4)
    nc.vector.tensor_add(c4, c4, m_t[:])
    tc_t = tcpool.tile([BL, 512], F32)
    nc.scalar.activation(tc_t[:], c_t[:], AF.Tanh)
    nc.vector.tensor_mul(h4, so, tc_t.rearrange("b (k x) -> b k x", k=4))

    # h^T for the next step: 4 PE transposes -> psum, one copy into hist.
    pt = ptr.tile([128, 64], F32)
    for k in range(4):
        nc.tensor.transpose(pt[:, 16 * k:16 * k + 16],
                            h_t[:, 128 * k:128 * (k + 1)], ident[:])
    nc.vector.tensor_copy(histv[:, :, slot_cur, :],
                          pt.rearrange("p (k b) -> p k b", k=4))


def _build(t_steps):
    ng = t_steps // GROUP
    nc = bacc.Bacc("TRN2", target_bir_lowering=False, debug=False,
                   enable_asserts=False, num_devices=NCORES)

    # I/O
    xloc = nc.dram_tensor("xloc", [BL, t_steps, F], F32, kind="ExternalInput")
    whhT_e = nc.dram_tensor("whhT_e", [H, 4 * H], F32, kind="ExternalInput")
    wxe_d = nc.dram_tensor("wxe", [F + 1, 4 * H], F32, kind="ExternalInput")
    wsumT_d = nc.dram_tensor("wsumT", [H, 4 * H], F32, kind="ExternalInput")
    wihT_d = nc.dram_tensor("wihT_d", [H, 4 * H], F32, kind="ExternalInput")
    bd_d = nc.dram_tensor("bd_row", [1, 4 * H], F32, kind="ExternalInput")
    wlinT_d = nc.dram_tensor("wlinT", [H, F], F32, kind="ExternalInput")
    blin_d = nc.dram_tensor("blin_rep", [128, F], F32, kind="ExternalInput")
    ident_d = nc.dram_tensor("ident", [BL, BL], F32, kind="ExternalInput")
    zo_d = nc.dram_tensor("zeros_ones", [129, 512], FR, kind="ExternalInput")
    proj_d = nc.dram_tensor("proj", [BL, t_steps, F], F32, kind="ExternalOutput")
    xlast_d = nc.dram_tensor("xlast", [BL, H], F32, kind="ExternalOutput")
    hdecT = nc.dram_tensor("hdecT", [ng * 128, 512], F32)  # staged h^T (decoder)

    with tile.TileContext(nc) as tc:
        with (
            tc.tile_pool(name="weights", bufs=1) as wpool,
            tc.tile_pool(name="state", bufs=1) as spool,
        ):
            whh_e = wpool.tile([128, 4, 4 * H], F32)
            wsum = wpool.tile([128, 4, 4 * H], F32)
            wihd = wpool.tile([128, 4, 4 * H], F32)
            wxe = wpool.tile([F + 1, 4 * H], F32)
            bd_t = wpool.tile([1, 4 * H], F32)
            wlin = wpool.tile([128, 4, F], F32)
            blin = wpool.tile([128, F], F32)
            ident = wpool.tile([BL, BL], F32)
            ones_t = wpool.tile([1, BL], F32)

            nc.sync.dma_start(whh_e[:], whhT_e.ap().rearrange("(k p) n -> p k n", p=128))
            nc.sync.dma_start(wsum[:], wsumT_d.ap().rearrange("(k p) n -> p k n", p=128))
            nc.sync.dma_start(wihd[:], wihT_d.ap().rearrange("(k p) n -> p k n", p=128))
            nc.sync.dma_start(wxe[:], wxe_d.ap())
            nc.sync.dma_start(bd_t[:], bd_d.ap())
            nc.sync.dma_start(wlin[:], wlinT_d.ap().rearrange("(k p) f -> p k f", p=128))
            nc.sync.dma_start(blin[:], blin_d.ap())
            nc.sync.dma_start(ident[:], ident_d.ap())
            nc.sync.dma_start(ones_t[:], zo_d.ap()[128:129, 0:BL])

            h_t = spool.tile([BL, H], F32)
            c_t = spool.tile([BL, H], F32)
            hist = spool.tile([128, 512], F32)   # cols = (k*8 + slot)*16 + b
            nc.gpsimd.memset(h_t[:], 0.0)
            nc.gpsimd.memset(c_t[:], 0.0)
            nc.sync.dma_start(hist[:], zo_d.ap()[0:128, :])

            state = dict(h=h_t, c=c_t, hist=hist, ident=ident, wxe=wxe)
            xv = xloc.ap().rearrange("b t f -> b (t f)")

            with (
                tc.tile_pool(name="pgates", bufs=2, space="PSUM") as pgates,
                tc.tile_pool(name="ptr", bufs=2, space="PSUM") as ptr,
                tc.tile_pool(name="xtp", bufs=1, space="PSUM") as xtp,
                tc.tile_pool(name="gpool", bufs=2) as gpool,
                tc.tile_pool(name="mpool", bufs=2) as mpool,
                tc.tile_pool(name="tcpool", bufs=2) as tcpool,
                tc.tile_pool(name="xraw", bufs=2) as xrawp,
                tc.tile_pool(name="xstage", bufs=2) as xsp,
                tc.tile_pool(name="pproj", bufs=1, space="PSUM") as pprojp,
            ):
                pools = dict(pgates=pgates, ptr=ptr, g=gpool, m=mpool, tc=tcpool)

                # ---- Encoder ----
                with tc.For_i(0, ng, 1) as j:
                    xraw = xrawp.tile([BL, GROUP * F], F32)
                    nc.sync.dma_start(xraw[:], xv[:, ds(j * (GROUP * F), GROUP * F)])
                    xps = xtp.tile([F, 128], F32)
                    for r in range(GROUP):
                        nc.tensor.transpose(xps[:, 16 * r:16 * r + 16],
                                            xraw[:, F * r:F * (r + 1)], ident[:])
                    xstage = xsp.tile([F + 1, 128], F32)
                    nc.vector.tensor_copy(xstage[0:F, :], xps[:])
                    nc.sync.dma_start(xstage[F:F + 1, :], zo_d.ap()[128:129, 0:128])
                    _emit_step_x(nc, pools, state, 0, xstage, None)
                    for r in range(GROUP):
                        nxt = (r + 1, xstage, None) if r < GROUP - 1 else None
                        _emit_step(nc, tc, r, pools, state, whh_e, xstage, None, nxt)

                # ---- Decoder ----
                nc.gpsimd.memset(c_t[:], 0.0)
                dec_bias = (ones_t, bd_t)
                # group 0: step 0 streams W_ih_dec against h_enc^T; rest W_sum
                def _emit_group_proj(col_ap):
                    pp = pprojp.tile([128, F], F32, name="pp", tag="pp")
                    for k in range(4):
                        nc.tensor.matmul(pp[:], _r(hist[:, 128 * k:128 * (k + 1)]),
                                         _r(wlin[:, k, :]),
                                         start=(k == 0), stop=(k == 3))
                    nc.vector.tensor_add(col_ap, pp[:], blin[:])

                _emit_step_x(nc, pools, state, 0, None, dec_bias)
                for r in range(GROUP):
                    mv = wihd if r == 0 else wsum
                    nxt = (r + 1, None, dec_bias) if r < GROUP - 1 else None
                    _emit_step(nc, tc, r, pools, state, mv, None, dec_bias, nxt)
                _emit_group_proj(proj_acc[:, 0:F])
                if ng > 2:
                    with tc.For_i(1, ng - 1, 1) as j:
                        _emit_step_x(nc, pools, state, 0, None, dec_bias)
                        for r in range(GROUP):
                            nxt = (r + 1, None, dec_bias) if r < GROUP - 1 else None
                            _emit_step(nc, tc, r, pools, state, wsum, None, dec_bias, nxt)
                        _emit_group_proj(proj_acc[:, ds(j * F, F)])
                if ng > 1:
                    # final group emitted statically: its adds anchor the
                    # post-loop copy behind every in-loop add (DVE in-order)
                    _emit_step_x(nc, pools, state, 0, None, dec_bias)
                    for r in range(GROUP):
                        nxt = (r + 1, None, dec_bias) if r < GROUP - 1 else None
                        _emit_step(nc, tc, r, pools, state, wsum, None, dec_bias, nxt)
                    _emit_group_proj(proj_acc[:, (ng - 1) * F:ng * F])

            tc.strict_bb_all_engine_barrier()
            nc.sync.dma_start(xlast_d.ap(), h_t[:])
            proj_out = spool.tile([128, ng * F], F32)
            nc.vector.tensor_copy(proj_out[:], proj_acc[:])
            if os.environ.get("KERNEL_DEBUG_PACC"):
                pdump = nc.dram_tensor("pdump", [128, ng * F], F32, kind="ExternalOutput")
                nc.sync.dma_start(pdump.ap(), proj_out[:])
            projv = proj_d.ap().rearrange("b (g s) f -> g s b f", s=GROUP)
            for j in range(ng):
                nc.sync.dma_start(projv[ng - 1 - j],
                                  proj_out[:, j * F:(j + 1) * F])

    nc.compile()
    return nc
def _prep_weights(inputs):
    perm = _gate_perm()
    f32 = lambda a: np.ascontiguousarray(np.asarray(a, dtype=np.float32))
    Wih_e = f32(inputs["W_ih_enc"])[perm]
    Whh_e = f32(inputs["W_hh_enc"])[perm]
    be = (f32(inputs["b_ih_enc"]) + f32(inputs["b_hh_enc"]))[perm]
    Wih_d = f32(inputs["W_ih_dec"])[perm]
    Whh_d = f32(inputs["W_hh_dec"])[perm]
    bd = (f32(inputs["b_ih_dec"]) + f32(inputs["b_hh_dec"]))[perm]
    Wsum = Wih_d + Whh_d
    return {
        "whhT_e": np.ascontiguousarray(Whh_e.T),
        "wxe": np.ascontiguousarray(np.vstack([Wih_e.T, be[None, :]])),
        "wsumT": np.ascontiguousarray(Wsum.T),
        "wihT_d": np.ascontiguousarray(Wih_d.T),
        "bd_row": np.ascontiguousarray(bd[None, :]),
        "wlinT": np.ascontiguousarray(f32(inputs["W_lin"]).T),
        "blin_rep": np.ascontiguousarray(np.tile(f32(inputs["b_lin"])[None, :], (128, 1))),
        "ident": np.eye(BL, dtype=np.float32),
        "zeros_ones": np.vstack([np.zeros((128, 512), np.float32),
                                 np.ones((1, 512), np.float32)]),
    }


def kernel(**inputs):
    global LAST_RESULTS
    x = np.ascontiguousarray(np.asarray(inputs["input_seq"], dtype=np.float32))
    t_steps = x.shape[1]
    if t_steps not in _CACHE:
        _CACHE[t_steps] = _build(t_steps)
    nc = _CACHE[t_steps]

    wmap = _prep_weights(inputs)
    in_maps = []
    for c in range(NCORES):
        m = dict(wmap)
        m["xloc"] = np.ascontiguousarray(x[c * BL:(c + 1) * BL])
        in_maps.append(m)

    trace = bool(int(os.environ.get("KERNEL_TRACE", "0")))
    if trace:
        _install_trace_hook()
    res = run_bass_kernel_spmd(nc, in_maps, list(range(NCORES)), trace=trace,
                               tmpdir=os.environ.get("KERNEL_TRACE_DIR"))
    LAST_RESULTS = res
    proj = np.concatenate([res.results[c]["proj"] for c in range(NCORES)], axis=0)
    xlast = np.concatenate([res.results[c]["xlast"] for c in range(NCORES)], axis=0)
    return proj, xlast
